# revision 36
# baseline (speedup 1.0000x reference)
"""Conv2d 3x3 (stride 1, pad 1) as implicit GEMM on 8 Trainium2 NeuronCores.

Problem: x [32,128,56,56] f32, weight [256,128,3,3] f32, bias [256] f32
         -> out [32,256,56,56] f32.

Sharding: data-parallel over batch. Each of the 8 cores gets 4 images;
weight/bias are replicated. No collectives; outputs are concatenated on host.

Per-core kernel (implicit GEMM, float32r matmuls):
  - x is host-padded to [4,128,58,58]; each image's padded plane lives in
    SBUF as a [128, 58, 58] tile (in-channels on partitions).
  - weight is host-rearranged to [128, 9, 256] (in-ch partitions, 3x3 taps,
    out-ch free) so lhsT slices need no on-device transpose.
  - For each image, out-channel group g (2 groups of 128) and band of 8
    output rows (7 bands): accumulate 9 matmuls (one per tap) into a
    [128, 448] PSUM tile: psum += W[:, ki, g*128:...].T @ xpad[:, rows+kh, kw:kw+56]
  - bias-add + PSUM->SBUF copy on the scalar engine, then DMA to DRAM.

Performance notes (measured on trn2 via NTFF/perfetto):
  - float32r streams 1 row/cycle at N>=256 (plain fp32 is 4 cycles/row):
    404us -> 132us.  Hardware rel err vs fp64-ish jax reference: 1.4e-4.
  - Matmul cadence is ~213ns for N=448 (186.7ns stream + ~26ns fixed issue
    overhead; measured independent of rhs AP shape and of LDWEIGHTS dedup).
  - The PE runs at ~99.8% occupancy between first and last matmul. The
    head is masked by fp32r warm-up matmuls on the first tiny DMA so the
    HAM clock-gate (1.2 -> 2.4 GHz) is warm before real work; input DMAs
    are split per row-band and interleaved with per-tap weight DMAs so the
    first bands' dependencies land one-transfer-per-queue.
"""

import numpy as np

import concourse.bacc as bacc
import concourse.mybir as mybir
import concourse.tile as tile
from concourse.bass_utils import run_bass_kernel_spmd

N_CORES = 8
B, C_IN, H, W = 32, 128, 56, 56
C_OUT = 256
KH = KW = 3
B_LOC = B // N_CORES          # 4 images per core
HP, WP = H + 2, W + 2         # 58 (pad=1)
ROWS = 8                      # output rows per matmul
NCHUNK = H // ROWS            # 7 bands
NFREE = ROWS * W              # 448 = matmul free dim (fits one PSUM bank)
NGRP = C_OUT // 128           # 2 out-channel groups

# float32r is the full-rate fp32 PE mode (1 cycle/row for N>=256 vs 4 for
# plain fp32). Flip to mybir.dt.float32 for bit-exact (but 3x slower) math.
MM_DT = mybir.dt.float32r


def _build():
    nc = bacc.Bacc(None, target_bir_lowering=False)
    xp = nc.dram_tensor("xp", [B_LOC, C_IN, HP, WP], MM_DT, kind="ExternalInput")
    wt = nc.dram_tensor("wt", [C_IN, KH * KW, C_OUT], MM_DT, kind="ExternalInput")
    bz = nc.dram_tensor("bz", [128, NGRP], mybir.dt.float32, kind="ExternalInput")
    out = nc.dram_tensor(
        "out", [B_LOC, NGRP, 128, H * W], mybir.dt.float32, kind="ExternalOutput"
    )

    with tile.TileContext(nc) as tc:
        with (
            tc.tile_pool(name="const", bufs=1) as cpool,
            tc.tile_pool(name="xin", bufs=B_LOC) as xpool,
            tc.tile_pool(name="oout", bufs=8) as opool,
            tc.tile_pool(name="psum", bufs=4, space="PSUM") as pspool,
        ):
            # PE warm-up: dummy fp32r matmuls on a small slice of real input,
            # loaded by the very first (tiny) DMA. Keeps the PE busy from
            # ~8us so the HAM clock-gate is at 8/8 and the fp32r pipeline is
            # primed before the first real matmul.
            wu = cpool.tile([128, ROWS, WP], MM_DT)
            nc.sync.dma_start(wu[:], xp[0, :, 0:ROWS])
            wu_ps = pspool.tile([128, NFREE], mybir.dt.float32, tag="warm", bufs=1)
            n_warm = 6
            for i in range(n_warm):
                nc.tensor.matmul(
                    wu_ps[:116],
                    wu[:, 0:2, 0:58],
                    wu[:, :, 0:W],
                    start=(i == 0),
                    stop=(i == n_warm - 1),
                )

            w_tile = cpool.tile([C_IN, KH * KW, C_OUT], MM_DT)
            b_tile = cpool.tile([128, NGRP], mybir.dt.float32)
            x_tiles = [
                xpool.tile([C_IN, HP, WP], MM_DT, name=f"x_img{b}", tag="ximg")
                for b in range(B_LOC)
            ]

            # chunk rc of image b: band-aligned row ranges. Band rc needs
            # padded rows [rc*ROWS, rc*ROWS+ROWS+2); chunk 0 covers rows
            # 0..9, chunk rc>=1 adds rows rc*ROWS+2 .. rc*ROWS+9.
            def load_chunk(b, rc):
                lo = 0 if rc == 0 else rc * ROWS + 2
                hi = rc * ROWS + ROWS + 2
                nc.sync.dma_start(x_tiles[b][:, lo:hi], xp[b, :, lo:hi])

            # DMA priority order, just-in-time for the first bands: image-0
            # band 0 + tap 0 (the first matmul's deps), then early chunks
            # interleaved with the remaining group-0 taps, bias, group-1
            # weights. One small transfer per DMA queue.
            load_chunk(0, 0)
            nc.sync.dma_start(w_tile[:, 0, 0:128], wt[:, 0, 0:128])
            load_chunk(0, 1)
            load_chunk(0, 2)
            load_chunk(0, 3)
            for ki in range(1, 5):
                nc.sync.dma_start(w_tile[:, ki, 0:128], wt[:, ki, 0:128])
            load_chunk(0, 4)
            load_chunk(0, 5)
            for ki in range(5, KH * KW):
                nc.sync.dma_start(w_tile[:, ki, 0:128], wt[:, ki, 0:128])
            load_chunk(0, 6)
            nc.sync.dma_start(b_tile[:], bz[:])
            for ki in range(KH * KW):
                nc.sync.dma_start(w_tile[:, ki, 128:256], wt[:, ki, 128:256])

            for b in range(B_LOC):
                for g in range(NGRP):
                    for rc in range(NCHUNK):
                        # trickle next image's chunks during the g=0 pass so
                        # prefetch doesn't starve this image's output DMAs
                        if g == 0 and b + 1 < B_LOC:
                            load_chunk(b + 1, rc)
                        ps = pspool.tile(
                            [128, NFREE], mybir.dt.float32, tag="ps", bufs=6
                        )
                        for ki in range(KH * KW):
                            kh, kw = divmod(ki, KW)
                            nc.tensor.matmul(
                                ps[:],
                                w_tile[:, ki, g * 128 : (g + 1) * 128],
                                x_tiles[b][
                                    :,
                                    rc * ROWS + kh : rc * ROWS + kh + ROWS,
                                    kw : kw + W,
                                ],
                                start=(ki == 0),
                                stop=(ki == KH * KW - 1),
                            )
                        o_tile = opool.tile(
                            [128, NFREE],
                            mybir.dt.float32,
                            name=f"o_{b}_{g}_{rc}",
                            tag="ot",
                        )
                        nc.scalar.activation(
                            o_tile[:],
                            ps[:],
                            mybir.ActivationFunctionType.Identity,
                            bias=b_tile[:, g : g + 1],
                            scale=1.0,
                        )
                        nc.sync.dma_start(
                            out[b, g, :, rc * NFREE : (rc + 1) * NFREE], o_tile[:]
                        )
    nc.finalize()
    return nc


_NC = None


def _prep_inputs(x, weight, bias):
    x = np.asarray(x, dtype=np.float32)
    weight = np.asarray(weight, dtype=np.float32)
    bias = np.asarray(bias, dtype=np.float32)
    xp = np.zeros((B, C_IN, HP, WP), dtype=np.float32)
    xp[:, :, 1 : H + 1, 1 : W + 1] = x
    # wt[p, kh*3+kw, o] = weight[o, p, kh, kw]
    wt = np.ascontiguousarray(
        weight.transpose(1, 2, 3, 0).reshape(C_IN, KH * KW, C_OUT)
    )
    # bz[p, g] = bias[g*128 + p]
    bz = np.ascontiguousarray(bias.reshape(NGRP, 128).T)
    return xp, wt, bz


def kernel(x, weight, bias, trace=False):
    global _NC
    xp, wt, bz = _prep_inputs(x, weight, bias)
    if _NC is None:
        _NC = _build()
    in_maps = [
        {"xp": xp[c * B_LOC : (c + 1) * B_LOC], "wt": wt, "bz": bz}
        for c in range(N_CORES)
    ]
    res = run_bass_kernel_spmd(
        _NC, in_maps, core_ids=list(range(N_CORES)), trace=trace
    )
    outs = [r["out"].reshape(B_LOC, C_OUT, H, W) for r in res.results]
    full = np.concatenate(outs, axis=0)
    if trace:
        return full, res
    return full
